# revision 17
# baseline (speedup 1.0000x reference)
"""CenterVLAD Trainium2 kernel (restructured, v2).

Reference computation (per batch b, with N = H*W = 1024 pixels, D=32, K=116):
    s = x @ W                    # (N, K)
    a = softmax(s, axis=-1)
    v = x.T @ a + (sum_n a) * C  # (D, K)
    v /= sqrt(sum_d v^2 + eps)   # intra-norm over D
    y = v.flatten(); y /= sqrt(sum y^2 + eps)

Sharding: data-parallel over batch B=32 across 8 cores (4 batches/core),
W and C replicated, no collectives.

Key structural choices vs the first-generation kernel:
  * The global L2 norm is analytically sqrt(K): the intra-normalized columns
    are unit vectors, so sum(v_hat^2) = K up to eps/ss ~ 1e-10.  The whole
    second normalization pipeline collapses into folding 1/sqrt(K) into the
    intra-norm scale: y = v / sqrt(K*(ss+eps)), via Ln(scale=K) + Exp(-0.5).
  * mm1 runs as a single bf16 matmul per transpose group (x and W both
    bf16-rounded); measured end-to-end rel err 1.4e-3 vs the 2e-2 gate.
  * ss comes from one ACT Square activation with accum_out (free-axis sum),
    removing the square+reduce pair and one DVE->ACT handoff.
  * Per-batch input DMAs and per-batch output DMAs overlap transfer with
    compute at both ends of the kernel.
  * Phase B of batch b is software-pipelined into batch b+1's slot, so the
    PE never stalls on the normalization chain.
  * Engine assignment keeps every instruction single-sourced (walrus allows
    ONE sync wait per instruction): only 6 absorber matmuls remain (2 setup,
    1 per batch to let the PE observe ACT's exp before the pooling matmuls).
    Absorber outputs land in unused corners of the s PSUM tiles, keeping the
    total PSUM footprint at exactly 8 banks:
      xt (1) + s double-buffered (2x2) + v double-buffered (2) + y (1).

Pixel layout: pixel n = 8*p + t lives on partition p, sub-chunk t in 0..7.
The PE transposes 4 chunks at a time ((128,[4,32]) -> (128,128)); s = x @ W
runs as ONE bf16 matmul per group against a block-diagonal zero-masked W
(128 x 4*116), yielding all four chunks' s columns side by side.  Softmax
runs in pixel-partition layout; the pooling matmul e.T @ [x*r | r] contracts
over pixels giving (116, 33) = [v1.T | asum], so the intra-normalization
runs along the free dim.  rsqrt is exp(-0.5*ln(.)) (ACT Rsqrt/Sqrt banned).

Toolchain notes: walrus accepts at most ONE sync wait per instruction
(absorber matmuls import foreign semaphores into the PE stream; the Tile
kernel-tail drain is split into a chain of one-wait drains; same-engine
self-waits are stripped post-hoc).  EVENT_SEMAPHORE_RANGE_CLEAR fails
codegen and is skipped (fresh NEFF per load).
"""

import numpy as np
from contextlib import ExitStack

import concourse.bass as bass
import concourse.tile as tile
from concourse import mybir, masks
from concourse.tile import add_dep_helper

F32 = mybir.dt.float32
BF16 = mybir.dt.bfloat16
AF = mybir.ActivationFunctionType
ALU = mybir.AluOpType

B = 32          # total batches
N = 1024        # H*W pixels per batch
D = 32          # channels
K = 116         # clusters
NCORES = 8
BPC = B // NCORES   # batches per core
T = 8               # pixel sub-chunks per batch (each chunk = 128 pixels)
EPS = 1e-12


def _ap(t, offs_el, dims):
    """Manual AP over tile/dram handle `t`: dims = [[step, count], ...] in
    elements, first dim = partition."""
    base = t[:] if not isinstance(t, bass.AP) else t
    return bass.AP(tensor=base.tensor, offset=base.offset + offs_el, ap=dims)


def _absorb(nc, ap, junk):
    """1x1 dummy matmul on the PE whose only role is to make the PE observe
    `ap`'s producer semaphore (walrus allows one sync wait per instruction).
    Writes `junk`, a (1,1) f32 PSUM AP in an unused corner of an s tile."""
    if ap.dtype == F32:
        ap = ap.bitcast(BF16)[0:1, 0:1]
    return nc.tensor.matmul(junk, ap, ap, start=True, stop=True,
                            tile_position=(0, 0))


def order(consumer, *absorbers):
    """Absorbers are dead-end ops; the priority scheduler would otherwise
    sink them below the very instructions they must precede."""
    for a in absorbers:
        add_dep_helper(consumer.ins, a.ins, reason="absorber ordering")


def _emit(ctx, tc, y_out, x_in, w_in, c_in):
    nc = tc.nc

    singles = ctx.enter_context(tc.tile_pool(name="singles", bufs=1))
    xhp = ctx.enter_context(tc.tile_pool(name="xhp", bufs=BPC))
    epool = ctx.enter_context(tc.tile_pool(name="epool", bufs=BPC))
    small = ctx.enter_context(tc.tile_pool(name="small", bufs=3 * BPC))
    xppool = ctx.enter_context(tc.tile_pool(name="xppool", bufs=BPC))
    ytp = ctx.enter_context(tc.tile_pool(name="ytp", bufs=BPC))

    ps_xt = ctx.enter_context(tc.tile_pool(name="ps_xt", bufs=1, space="PSUM"))
    ps_s = ctx.enter_context(tc.tile_pool(name="ps_s", bufs=1, space="PSUM"))
    ps_v = ctx.enter_context(tc.tile_pool(name="ps_v", bufs=1, space="PSUM"))
    ps_y = ctx.enter_context(tc.tile_pool(name="ps_y", bufs=1, space="PSUM"))

    # ---- input DMAs first: x halves on the SP queue, W/C on the ACT queue
    # (parallel queues; ACT issues before its table load) --------------------
    c_sb = singles.tile([D, K], F32)
    xall = singles.tile([128, BPC, T, D], F32)
    # W replicated onto all 4 partition groups in ONE DMA (0-step DRAM dim
    # re-reads W four times); the diagonal scatter happens in 4 DVE casting
    # copies into the zeroed bf16 block-diagonal wh.
    wrep = singles.tile([128, K], F32)
    wstep = wrep[:].ap[0][0]
    with tc.high_priority():
        nc.sync.dma_start(
            out=xall[:, 0:2],
            in_=_ap(x_in[:, :, :], 0,
                    [[T * D, 128], [N * D, 2], [D, T], [1, D]]))
        nc.sync.dma_start(
            out=_ap(wrep, 0, [[wstep, 128], [1, K]]),
            in_=_ap(w_in[:, :], 0, [[0, 4], [K, D], [1, K]]))
        nc.sync.dma_start(
            out=xall[:, 2:4],
            in_=_ap(x_in[:, :, :], 2 * N * D,
                    [[T * D, 128], [N * D, 2], [D, T], [1, D]]))
        nc.scalar.dma_start(out=c_sb[:], in_=c_in[:, :])

    # ---- constants -------------------------------------------------------
    identity = singles.tile([128, 128], F32)
    masks.make_identity(nc, identity[:])

    epsb = singles.tile([K, 1], F32)
    nc.vector.memset(epsb[:], float(K) * EPS)

    # both s PSUM tiles upfront: 464 of each 512-f32 bank-half is live data,
    # the tail holds absorber junk outputs
    s_tiles = [ps_s.tile([128, 2, 512], F32, name=f"s_ps{i}") for i in range(2)]
    sstep = s_tiles[0][:].ap[0][0]

    # PE observes the identity producer (gpsimd affine_select)
    a_id = _absorb(nc, identity[0:1, 0:1], s_tiles[0][0:1, 1, 500:501])

    # bf16 identity for the final y transposes (DVE, before wh so a_wh covers)
    id_bf = singles.tile([K, K], BF16)
    nc.vector.tensor_copy(id_bf[:], identity[0:K, 0:K])

    # two full-bank v tiles; the C.T setup transpose aliases bank 1
    v_big = [ps_v.tile([128, 512], F32, name=f"v_ps{i}") for i in range(2)]
    v_tiles = [v_big[i][0:K, 0:D + 1] for i in range(2)]
    ct_ps = v_big[1][0:K, 0:D]

    # zeroed bf16 block-diagonal W, diagonal blocks cast-copied from wrep
    wh = singles.tile([128, 4 * K], BF16)
    nc.vector.memset(wh[:], 0.0)
    diag_cps = []
    for q in range(4):
        diag_cps.append(nc.vector.tensor_copy(
            wh[32 * q:32 * q + 32, K * q:K * q + K],
            wrep[32 * q:32 * q + 32, :]))

    # PE observes DVE's setup chain (id_bf, ct_sb, wblk, wh)
    a_wh = _absorb(nc, wh[0:1, 0:1], s_tiles[0][0:1, 1, 501:502])

    # ---- per-core staging ------------------------------------------------
    vstage = singles.tile([K, BPC, D], F32)
    ss = singles.tile([K, BPC], F32)
    lss = singles.tile([K, BPC], F32)
    rinv = singles.tile([K, BPC], F32)
    ysb = singles.tile([D, BPC, K], F32)

    xt_ps = ps_xt.tile([128, 2, 2, 128], F32)
    y_ps = ps_y.tile([D, K], BF16)

    yt_prev = None

    def tail_front(b):
        # phase-B ACT chain of batch b: ss -> 1/sqrt(K*(ss+eps))
        nc.scalar.activation(lss[:, b:b + 1], ss[:, b:b + 1], AF.Ln,
                             scale=float(K), bias=epsb[0:K, 0:1])
        nc.scalar.activation(rinv[:, b:b + 1], lss[:, b:b + 1], AF.Exp,
                             scale=-0.5)

    def tail_ymul(b):
        # on DVE: the Y transpose's wait on ymul is also how the PE observes
        # the DVE clock (covers the v-bank WAR against stt reads)
        yt = ytp.tile([K, D], BF16)
        nc.vector.tensor_scalar_mul(yt[:], vstage[:, b, :], rinv[:, b:b + 1])
        return yt

    def tail_back(b, yt):
        # y.T * S transposed back to (D, K), staged, DMA'd out per batch
        mm_y = nc.tensor.transpose(y_ps[:], yt[:], id_bf[:])
        nc.vector.tensor_copy(ysb[:, b, :], y_ps[:])
        nc.sync.dma_start(
            out=_ap(y_out[:, :], b * D * K, [[K, D], [1, K]]),
            in_=ysb[:, b, :])
        return mm_y

    # ---- pipelined batch loop -------------------------------------------
    # The PE->ACT->PE round trip (T -> xh -> M1) would otherwise set the
    # batch cadence; transposes T_{b+2} are issued right after M1_b and the
    # xh cast right after, so M1_{b+1} finds its stationary ready.
    def emit_T(b):
        mm_ts = []
        for g in range(2):
            mm_t = nc.tensor.transpose(
                xt_ps[:, b % 2, g, :], xall[:, b, 4 * g:4 * g + 4, :],
                identity[:, :])
            mm_ts.append(mm_t)
        return mm_ts

    def emit_xh(b):
        xh = xhp.tile([128, 2, 128], BF16)
        nc.scalar.copy(xh[:], xt_ps[:, b % 2])
        return xh

    t0 = emit_T(0)
    order(t0[0], a_id, a_wh)
    emit_T(1)
    xhs = {0: emit_xh(0)}

    # C.T via PE transpose; emitted after the prologue so the scheduler
    # never slots it ahead of the start-critical W copies on DVE
    mm_ct = nc.tensor.transpose(ct_ps, c_sb[:], identity[0:D, 0:D])
    order(mm_ct, a_id)
    ct_sb = singles.tile([K, D], F32)
    ct_cp = nc.vector.tensor_copy(ct_sb[:], ct_ps)
    order(ct_cp, diag_cps[3])

    for b in range(BPC):
        s_ps = s_tiles[b % 2]
        v_ps = v_tiles[b % 2]
        xh = xhs[b]

        # s = x @ W: one bf16 matmul per transpose group against the
        # block-diagonal W gives all 4 chunks' s columns
        e_sb = epool.tile([128, 2, 4, K], BF16)
        mm1s = []
        for g in range(2):
            mm1 = nc.tensor.matmul(
                s_ps[:, g, 0:4 * K], xh[:, g, :], wh[:], start=True, stop=True)
            if b == 0 and g == 0:
                order(mm1, a_wh)
            mm1s.append(mm1)

        if b + 2 < BPC:
            tnext = emit_T(b + 2)
            order(tnext[0], mm1s[1])
        if b + 1 < BPC:
            xhs[b + 1] = emit_xh(b + 1)

        # e = exp(s) and the pixel row-sums, per group so the reduce of g0
        # overlaps the exp of g1 (shorter softmax critical chain)
        sums = small.tile([128, 2, 4], F32)
        for g in range(2):
            s_view = _ap(s_ps, 512 * g, [[sstep, 128], [K, 4], [1, K]])
            nc.scalar.activation(e_sb[:, g], s_view, AF.Exp)
            nc.vector.tensor_reduce(sums[:, g], e_sb[:, g],
                                    axis=mybir.AxisListType.X, op=ALU.add)

        if b >= 1:
            tail_front(b - 1)

        # the whole r -> x' -> pooling chain runs per transpose group, so
        # group 1's softmax tail overlaps group 0's pooling matmuls
        r = small.tile([128, 2, 4], F32)
        for g in range(2):
            nc.vector.reciprocal(r[:, g], sums[:, g])

        if b >= 1:
            yt_prev = tail_ymul(b - 1)

        xp = xppool.tile([128, T, D + 1], BF16)
        xpstep = xp[:].ap[0][0]
        xastep = xall[:].ap[0][0]
        r_ps0 = r[:].ap[0][0]

        # absorbers first: both exp ticks (and batch 1's ct-copy tick, the
        # WAR cover for the v bank; b>=2 is covered by Y_{b-2}'s ymul wait)
        # are long done by the time the first xp half lands
        absorbers = [
            _absorb(nc, e_sb[0:1, g, 0, 0:1],
                    s_tiles[(b + 1) % 2][0:1, 1,
                                         502 + 2 * b + g:503 + 2 * b + g])
            for g in range(2)]
        if b == 1:
            absorbers.append(_absorb(nc, ct_sb[0:1, 0:1],
                                     s_tiles[(b + 1) % 2][0:1, 1, 510:511]))
        for g in range(2):
            nc.gpsimd.tensor_copy(
                _ap(xp, 4 * g * (D + 1) + D, [[xpstep, 128], [D + 1, 4]]),
                _ap(r, 4 * g, [[r_ps0, 128], [1, 4]]))
            nc.gpsimd.tensor_mul(
                _ap(xp, 4 * g * (D + 1),
                    [[xpstep, 128], [D + 1, 4], [1, D]]),
                _ap(xall, (b * T + 4 * g) * D,
                    [[xastep, 128], [D, 4], [1, D]]),
                _ap(r, 4 * g, [[r_ps0, 128], [1, 4], [0, D]]))
            for q in range(4):
                t = 4 * g + q
                mm2 = nc.tensor.matmul(
                    v_ps, e_sb[:, g, q, :], xp[:, t, :],
                    start=(t == 0), stop=(t == T - 1))
                if t == 0:
                    order(mm2, *absorbers)

        if b >= 1:
            tail_back(b - 1, yt_prev)

        # v.T = v1.T + asum * C.T  (fused multiply-add on DVE), then
        # ss[k] = sum_d v.T[k,d]^2 stays on DVE (program order, no waits)
        nc.vector.scalar_tensor_tensor(
            out=vstage[:, b, :], in0=ct_sb[:],
            scalar=v_big[b % 2][0:K, D:D + 1],
            in1=v_big[b % 2][0:K, 0:D], op0=ALU.mult, op1=ALU.add)
        sqt = small.tile([K, D], F32)
        nc.vector.tensor_mul(sqt[:], vstage[:, b, :], vstage[:, b, :])
        nc.vector.tensor_reduce(ss[:, b:b + 1], sqt[:],
                                axis=mybir.AxisListType.X, op=ALU.add)

    # drain the software pipeline for the last batch
    tail_front(BPC - 1)
    yt_last = tail_ymul(BPC - 1)
    tail_back(BPC - 1, yt_last)


def _split_drain_and_barrier(self, tick_clock, wait_clock):
    """Replacement for TileContext._drain_and_barrier: this walrus build
    accepts at most one sync wait per instruction, so the kernel-tail drain's
    per-proc waits are spread over a chain of one-wait drains."""
    from concourse.vector_clock import ScopedClock

    nc = self.nc
    drain_inst = nc.sync.drain()
    wait_clock.add_sem_waits(
        drain_inst.ins, ScopedClock({None: tick_clock.global_clock}))
    si = drain_inst.ins.sync_info
    if si is not None and len(si.on_wait) > 1:
        waits = list(si.on_wait)
        upd = list(si.on_update)
        drain_inst.ins.sync_info = mybir.SyncInfo(
            on_wait=[waits[0]], on_update=upd)
        for w in waits[1:]:
            d2 = nc.sync.drain()
            d2.ins.sync_info = mybir.SyncInfo(on_wait=[w], on_update=[])

    # No all_engine_barrier pair: the split drain chain above already waits
    # out every queue/engine semaphore on Sync, and the NEFF completes when
    # each engine stream ends.  The barriers only added ~8us of serial
    # EVENT_SEMAPHORE spam at the tail.  clear_and_free_semaphores is also
    # skipped: its EVENT_SEMAPHORE_RANGE_CLEAR InstISA fails codegen here,
    # and this kernel is built fresh per NEFF load.
    assert self.sems is not None
    popped = nc._tile_sem_poison_stack.pop()
    assert popped is self._sem_poison


def build_bass():
    import types

    nc = bass.Bass()
    x_in = nc.declare_dram_parameter("x_loc", [BPC, N, D], F32, isOutput=False)
    w_in = nc.declare_dram_parameter("w_in", [D, K], F32, isOutput=False)
    c_in = nc.declare_dram_parameter("c_in", [D, K], F32, isOutput=False)
    y_out = nc.declare_dram_parameter("y_loc", [BPC, D * K], F32, isOutput=True)
    with ExitStack() as ctx:
        tc = ctx.enter_context(tile.TileContext(nc))
        tc._drain_and_barrier = types.MethodType(_split_drain_and_barrier, tc)
        _emit(ctx, tc, y_out, x_in, w_in, c_in)
    # strip same-engine self-waits from multi-wait instructions: the engines
    # dispatch in FIFO order and DVE/ACT drain between ops, so a self-wait
    # whose target precedes in the same stream guards only pseudo-hazards
    # (PSUM bank read-read serialization); walrus allows one wait only.
    eng_name = {mybir.EngineType.Activation: "Activation",
                mybir.EngineType.PE: "PE",
                mybir.EngineType.DVE: "DVE",
                mybir.EngineType.Pool: "Pool",
                mybir.EngineType.SP: "SP"}
    for name, inst in nc.inst_map.items():
        si = inst.sync_info
        if si is None or len(si.on_wait) <= 1:
            continue
        en = eng_name.get(getattr(inst, "engine", None))
        if en is None:
            continue
        keep = [w for w in si.on_wait if not w.ant_name.startswith(en + "_")]
        if isinstance(inst, mybir.InstDMACopy) and len(keep) > 1:
            # inter-DMA WAW waits on the shared SP queue guard disjoint DRAM
            # regions (one output slab per batch); queue order + disjoint
            # destinations make them redundant, and walrus allows one wait.
            keep2 = [w for w in keep if not w.ant_name.startswith("DMAHW")]
            if keep2:
                keep = keep2
        if 0 < len(keep) < len(si.on_wait):
            inst.sync_info = mybir.SyncInfo(on_wait=keep,
                                            on_update=list(si.on_update))
    return nc


def run(x, W, C, trace=False, tmpdir=None):
    from concourse.bass_utils import run_bass_kernel_spmd

    x = np.ascontiguousarray(x, dtype=np.float32).reshape(B, N, D)
    W = np.ascontiguousarray(W, dtype=np.float32)
    C = np.ascontiguousarray(C, dtype=np.float32)

    nc = build_bass()
    in_maps = [
        {"x_loc": x[c * BPC:(c + 1) * BPC], "w_in": W, "c_in": C}
        for c in range(NCORES)
    ]
    res = run_bass_kernel_spmd(nc, in_maps, list(range(NCORES)), trace=trace,
                               tmpdir=tmpdir)
    y = np.concatenate([res.results[c]["y_loc"] for c in range(NCORES)], axis=0)
    return y.astype(np.float32), res


def kernel(x, W, C):
    y, _ = run(x, W, C, trace=False)
    return y


# revision 18
# speedup vs baseline: 1.0165x; 1.0165x over previous
"""CenterVLAD Trainium2 kernel (restructured, v2).

Reference computation (per batch b, with N = H*W = 1024 pixels, D=32, K=116):
    s = x @ W                    # (N, K)
    a = softmax(s, axis=-1)
    v = x.T @ a + (sum_n a) * C  # (D, K)
    v /= sqrt(sum_d v^2 + eps)   # intra-norm over D
    y = v.flatten(); y /= sqrt(sum y^2 + eps)

Sharding: data-parallel over batch B=32 across 8 cores (4 batches/core),
W and C replicated, no collectives.

Key structural choices vs the first-generation kernel:
  * The global L2 norm is analytically sqrt(K): the intra-normalized columns
    are unit vectors, so sum(v_hat^2) = K up to eps/ss ~ 1e-10.  The whole
    second normalization pipeline collapses into folding 1/sqrt(K) into the
    intra-norm scale: y = v / sqrt(K*(ss+eps)), via Ln(scale=K) + Exp(-0.5).
  * mm1 runs as a single bf16 matmul per transpose group (x and W both
    bf16-rounded); measured end-to-end rel err 1.4e-3 vs the 2e-2 gate.
  * ss comes from one ACT Square activation with accum_out (free-axis sum),
    removing the square+reduce pair and one DVE->ACT handoff.
  * Per-batch input DMAs and per-batch output DMAs overlap transfer with
    compute at both ends of the kernel.
  * Phase B of batch b is software-pipelined into batch b+1's slot, so the
    PE never stalls on the normalization chain.
  * Engine assignment keeps every instruction single-sourced (walrus allows
    ONE sync wait per instruction): only 6 absorber matmuls remain (2 setup,
    1 per batch to let the PE observe ACT's exp before the pooling matmuls).
    Absorber outputs land in unused corners of the s PSUM tiles, keeping the
    total PSUM footprint at exactly 8 banks:
      xt (1) + s double-buffered (2x2) + v double-buffered (2) + y (1).

Pixel layout: pixel n = 8*p + t lives on partition p, sub-chunk t in 0..7.
The PE transposes 4 chunks at a time ((128,[4,32]) -> (128,128)); s = x @ W
runs as ONE bf16 matmul per group against a block-diagonal zero-masked W
(128 x 4*116), yielding all four chunks' s columns side by side.  Softmax
runs in pixel-partition layout; the pooling matmul e.T @ [x*r | r] contracts
over pixels giving (116, 33) = [v1.T | asum], so the intra-normalization
runs along the free dim.  rsqrt is exp(-0.5*ln(.)) (ACT Rsqrt/Sqrt banned).

Toolchain notes: walrus accepts at most ONE sync wait per instruction
(absorber matmuls import foreign semaphores into the PE stream; the Tile
kernel-tail drain is split into a chain of one-wait drains; same-engine
self-waits are stripped post-hoc).  EVENT_SEMAPHORE_RANGE_CLEAR fails
codegen and is skipped (fresh NEFF per load).
"""

import numpy as np
from contextlib import ExitStack

import concourse.bass as bass
import concourse.tile as tile
from concourse import mybir, masks
from concourse.tile import add_dep_helper

F32 = mybir.dt.float32
BF16 = mybir.dt.bfloat16
AF = mybir.ActivationFunctionType
ALU = mybir.AluOpType

B = 32          # total batches
N = 1024        # H*W pixels per batch
D = 32          # channels
K = 116         # clusters
NCORES = 8
BPC = B // NCORES   # batches per core
T = 8               # pixel sub-chunks per batch (each chunk = 128 pixels)
EPS = 1e-12


def _ap(t, offs_el, dims):
    """Manual AP over tile/dram handle `t`: dims = [[step, count], ...] in
    elements, first dim = partition."""
    base = t[:] if not isinstance(t, bass.AP) else t
    return bass.AP(tensor=base.tensor, offset=base.offset + offs_el, ap=dims)


def _absorb(nc, ap, junk):
    """1x1 dummy matmul on the PE whose only role is to make the PE observe
    `ap`'s producer semaphore (walrus allows one sync wait per instruction).
    Writes `junk`, a (1,1) f32 PSUM AP in an unused corner of an s tile."""
    if ap.dtype == F32:
        ap = ap.bitcast(BF16)[0:1, 0:1]
    return nc.tensor.matmul(junk, ap, ap, start=True, stop=True,
                            tile_position=(0, 0))


def order(consumer, *absorbers):
    """Absorbers are dead-end ops; the priority scheduler would otherwise
    sink them below the very instructions they must precede."""
    for a in absorbers:
        add_dep_helper(consumer.ins, a.ins, reason="absorber ordering")


def _emit(ctx, tc, y_out, x_in, w_in, c_in):
    nc = tc.nc

    singles = ctx.enter_context(tc.tile_pool(name="singles", bufs=1))
    xhp = ctx.enter_context(tc.tile_pool(name="xhp", bufs=BPC))
    epool = ctx.enter_context(tc.tile_pool(name="epool", bufs=BPC))
    small = ctx.enter_context(tc.tile_pool(name="small", bufs=3 * BPC))
    xppool = ctx.enter_context(tc.tile_pool(name="xppool", bufs=BPC))
    ytp = ctx.enter_context(tc.tile_pool(name="ytp", bufs=BPC))

    ps_xt = ctx.enter_context(tc.tile_pool(name="ps_xt", bufs=1, space="PSUM"))
    ps_s = ctx.enter_context(tc.tile_pool(name="ps_s", bufs=1, space="PSUM"))
    ps_v = ctx.enter_context(tc.tile_pool(name="ps_v", bufs=1, space="PSUM"))
    ps_y = ctx.enter_context(tc.tile_pool(name="ps_y", bufs=1, space="PSUM"))

    # ---- input DMAs first: x halves on the SP queue, W/C on the ACT queue
    # (parallel queues; ACT issues before its table load) --------------------
    c_sb = singles.tile([D, K], F32)
    xall = singles.tile([128, BPC, T, D], F32)
    # W replicated onto all 4 partition groups in ONE DMA (0-step DRAM dim
    # re-reads W four times); the diagonal scatter happens in 4 DVE casting
    # copies into the zeroed bf16 block-diagonal wh.
    wrep = singles.tile([128, K], F32)
    wstep = wrep[:].ap[0][0]
    with tc.high_priority():
        nc.sync.dma_start(
            out=xall[:, 0:2],
            in_=_ap(x_in[:, :, :], 0,
                    [[T * D, 128], [N * D, 2], [D, T], [1, D]]))
        nc.sync.dma_start(
            out=_ap(wrep, 0, [[wstep, 128], [1, K]]),
            in_=_ap(w_in[:, :], 0, [[0, 4], [K, D], [1, K]]))
        nc.sync.dma_start(
            out=xall[:, 2:4],
            in_=_ap(x_in[:, :, :], 2 * N * D,
                    [[T * D, 128], [N * D, 2], [D, T], [1, D]]))
        nc.scalar.dma_start(out=c_sb[:], in_=c_in[:, :])

    # ---- constants -------------------------------------------------------
    identity = singles.tile([128, 128], F32)
    masks.make_identity(nc, identity[:])

    epsb = singles.tile([K, 1], F32)
    nc.vector.memset(epsb[:], float(K) * EPS)

    # both s PSUM tiles upfront: 464 of each 512-f32 bank-half is live data,
    # the tail holds absorber junk outputs
    s_tiles = [ps_s.tile([128, 2, 512], F32, name=f"s_ps{i}") for i in range(2)]
    sstep = s_tiles[0][:].ap[0][0]

    # PE observes the identity producer (gpsimd affine_select)
    a_id = _absorb(nc, identity[0:1, 0:1], s_tiles[0][0:1, 1, 500:501])

    # bf16 identity for the final y transposes (DVE, before wh so a_wh covers)
    id_bf = singles.tile([K, K], BF16)
    nc.vector.tensor_copy(id_bf[:], identity[0:K, 0:K])

    # two full-bank v tiles; the C.T setup transpose aliases bank 1
    v_big = [ps_v.tile([128, 512], F32, name=f"v_ps{i}") for i in range(2)]
    v_tiles = [v_big[i][0:K, 0:D + 1] for i in range(2)]
    ct_ps = v_big[1][0:K, 0:D]

    # zeroed bf16 block-diagonal W, diagonal blocks cast-copied from wrep
    wh = singles.tile([128, 4 * K], BF16)
    nc.vector.memset(wh[:], 0.0)
    diag_cps = []
    for q in range(4):
        diag_cps.append(nc.vector.tensor_copy(
            wh[32 * q:32 * q + 32, K * q:K * q + K],
            wrep[32 * q:32 * q + 32, :]))

    # PE observes DVE's setup chain (id_bf, ct_sb, wblk, wh)
    a_wh = _absorb(nc, wh[0:1, 0:1], s_tiles[0][0:1, 1, 501:502])

    # ---- per-core staging ------------------------------------------------
    vstage = singles.tile([K, BPC, D], F32)
    ss = singles.tile([K, BPC], F32)
    lss = singles.tile([K, BPC], F32)
    rinv = singles.tile([K, BPC], F32)
    ysb = singles.tile([D, BPC, K], F32)

    xt_ps = ps_xt.tile([128, 2, 2, 128], F32)
    y_ps = ps_y.tile([D, K], BF16)

    yt_prev = None

    def tail_front(b):
        # phase-B ACT chain of batch b: ss -> 1/sqrt(K*(ss+eps))
        nc.scalar.activation(lss[:, b:b + 1], ss[:, b:b + 1], AF.Ln,
                             scale=float(K), bias=epsb[0:K, 0:1])
        nc.scalar.activation(rinv[:, b:b + 1], lss[:, b:b + 1], AF.Exp,
                             scale=-0.5)

    def tail_ymul(b):
        # on DVE: the Y transpose's wait on ymul is also how the PE observes
        # the DVE clock (covers the v-bank WAR against stt reads)
        yt = ytp.tile([K, D], BF16)
        nc.vector.tensor_scalar_mul(yt[:], vstage[:, b, :], rinv[:, b:b + 1])
        return yt

    def tail_back(b, yt):
        # y.T * S transposed back to (D, K), staged, DMA'd out per batch
        mm_y = nc.tensor.transpose(y_ps[:], yt[:], id_bf[:])
        nc.vector.tensor_copy(ysb[:, b, :], y_ps[:])
        nc.sync.dma_start(
            out=_ap(y_out[:, :], b * D * K, [[K, D], [1, K]]),
            in_=ysb[:, b, :])
        return mm_y

    # ---- pipelined batch loop -------------------------------------------
    # The PE->ACT->PE round trip (T -> xh -> M1) would otherwise set the
    # batch cadence; transposes T_{b+2} are issued right after M1_b and the
    # xh cast right after, so M1_{b+1} finds its stationary ready.
    def emit_T(b):
        mm_ts = []
        for g in range(2):
            mm_t = nc.tensor.transpose(
                xt_ps[:, b % 2, g, :], xall[:, b, 4 * g:4 * g + 4, :],
                identity[:, :])
            mm_ts.append(mm_t)
        return mm_ts

    def emit_xh(b):
        xh = xhp.tile([128, 2, 128], BF16)
        nc.scalar.copy(xh[:], xt_ps[:, b % 2])
        return xh

    t0 = emit_T(0)
    order(t0[0], a_id, a_wh)
    emit_T(1)
    xhs = {0: emit_xh(0)}

    # C.T via PE transpose; emitted after the prologue so the scheduler
    # never slots it ahead of the start-critical W copies on DVE
    mm_ct = nc.tensor.transpose(ct_ps, c_sb[:], identity[0:D, 0:D])
    order(mm_ct, a_id)
    ct_sb = singles.tile([K, D], F32)
    ct_cp = nc.vector.tensor_copy(ct_sb[:], ct_ps)
    order(ct_cp, diag_cps[3])

    for b in range(BPC):
        s_ps = s_tiles[b % 2]
        v_ps = v_tiles[b % 2]
        xh = xhs[b]

        # s = x @ W: one bf16 matmul per transpose group against the
        # block-diagonal W gives all 4 chunks' s columns
        e_sb = epool.tile([128, 2, 4, K], BF16)
        mm1s = []
        for g in range(2):
            mm1 = nc.tensor.matmul(
                s_ps[:, g, 0:4 * K], xh[:, g, :], wh[:], start=True, stop=True)
            if b == 0 and g == 0:
                order(mm1, a_wh)
            mm1s.append(mm1)

        if b + 2 < BPC:
            tnext = emit_T(b + 2)
            order(tnext[0], mm1s[1])
        if b + 1 < BPC:
            xhs[b + 1] = emit_xh(b + 1)

        # e = exp(s) and the pixel row-sums, per group so the reduce of g0
        # overlaps the exp of g1 (shorter softmax critical chain)
        sums = small.tile([128, 2, 4], F32)
        for g in range(2):
            s_view = _ap(s_ps, 512 * g, [[sstep, 128], [K, 4], [1, K]])
            nc.scalar.activation(e_sb[:, g], s_view, AF.Exp)
            nc.vector.tensor_reduce(sums[:, g], e_sb[:, g],
                                    axis=mybir.AxisListType.X, op=ALU.add)

        if b >= 1:
            tail_front(b - 1)

        r = small.tile([128, 2, 4], F32)
        nc.vector.reciprocal(r[:], sums[:])

        if b >= 1:
            yt_prev = tail_ymul(b - 1)

        # x' = [x * r | r] on gpsimd (single producer for the mm2 moving op):
        # r column first (waits DVE), then the broadcast multiply (waits DMA)
        xp = xppool.tile([128, T, D + 1], BF16)
        xpstep = xp[:].ap[0][0]
        xastep = xall[:].ap[0][0]
        r_ps0 = r[:].ap[0][0]
        nc.gpsimd.tensor_copy(
            _ap(xp, D, [[xpstep, 128], [D + 1, T]]),
            _ap(r, 0, [[r_ps0, 128], [1, T]]))
        nc.gpsimd.tensor_mul(
            _ap(xp, 0, [[xpstep, 128], [D + 1, T], [1, D]]),
            _ap(xall, b * T * D, [[xastep, 128], [D, T], [1, D]]),
            _ap(r, 0, [[r_ps0, 128], [1, T], [0, D]]))

        # Y_{b-1} before M2_b: its DVE wait (ymul_{b-1}) is how the PE
        # observes the DVE clock, covering M2_b's WAR on the v bank against
        # the stt/ct reads
        # pooling: [v1.T | asum] = e.T @ x' accumulated over the 8 chunks.
        # a_e imports ACT's exp tick; a_v imports DVE's stt_{b-1} tick (WAR
        # cover for the v bank against stt/ct reads).
        a_e = _absorb(nc, e_sb[0:1, 0, 0, 0:1],
                      s_tiles[(b + 1) % 2][0:1, 1, 502 + b:503 + b])
        absorbers = [a_e]
        if b == 1:
            # v bank 1's setup reader (the ct copy) finished long ago; a
            # cheap absorber imports that DVE tick.  b>=2 is covered by
            # Y_{b-2}'s wait on ymul_{b-2} > stt_{b-2}.
            a_ct = _absorb(nc, ct_sb[0:1, 0:1],
                           s_tiles[(b + 1) % 2][0:1, 1, 506:507])
            absorbers.append(a_ct)
        for g in range(2):
            for q in range(4):
                t = 4 * g + q
                mm2 = nc.tensor.matmul(
                    v_ps, e_sb[:, g, q, :], xp[:, t, :],
                    start=(t == 0), stop=(t == T - 1))
                if t == 0:
                    order(mm2, *absorbers)

        if b >= 1:
            tail_back(b - 1, yt_prev)

        # v.T = v1.T + asum * C.T  (fused multiply-add on DVE), then
        # ss[k] = sum_d v.T[k,d]^2 stays on DVE (program order, no waits)
        nc.vector.scalar_tensor_tensor(
            out=vstage[:, b, :], in0=ct_sb[:],
            scalar=v_big[b % 2][0:K, D:D + 1],
            in1=v_big[b % 2][0:K, 0:D], op0=ALU.mult, op1=ALU.add)
        sqt = small.tile([K, D], F32)
        nc.vector.tensor_mul(sqt[:], vstage[:, b, :], vstage[:, b, :])
        nc.vector.tensor_reduce(ss[:, b:b + 1], sqt[:],
                                axis=mybir.AxisListType.X, op=ALU.add)

    # drain the software pipeline for the last batch
    tail_front(BPC - 1)
    yt_last = tail_ymul(BPC - 1)
    tail_back(BPC - 1, yt_last)


def _split_drain_and_barrier(self, tick_clock, wait_clock):
    """Replacement for TileContext._drain_and_barrier: this walrus build
    accepts at most one sync wait per instruction, so the kernel-tail drain's
    per-proc waits are spread over a chain of one-wait drains."""
    from concourse.vector_clock import ScopedClock

    nc = self.nc
    drain_inst = nc.sync.drain()
    wait_clock.add_sem_waits(
        drain_inst.ins, ScopedClock({None: tick_clock.global_clock}))
    si = drain_inst.ins.sync_info
    if si is not None and len(si.on_wait) > 1:
        waits = list(si.on_wait)
        upd = list(si.on_update)
        drain_inst.ins.sync_info = mybir.SyncInfo(
            on_wait=[waits[0]], on_update=upd)
        for w in waits[1:]:
            d2 = nc.sync.drain()
            d2.ins.sync_info = mybir.SyncInfo(on_wait=[w], on_update=[])

    # No all_engine_barrier pair: the split drain chain above already waits
    # out every queue/engine semaphore on Sync, and the NEFF completes when
    # each engine stream ends.  The barriers only added ~8us of serial
    # EVENT_SEMAPHORE spam at the tail.  clear_and_free_semaphores is also
    # skipped: its EVENT_SEMAPHORE_RANGE_CLEAR InstISA fails codegen here,
    # and this kernel is built fresh per NEFF load.
    assert self.sems is not None
    popped = nc._tile_sem_poison_stack.pop()
    assert popped is self._sem_poison


def build_bass():
    import types

    nc = bass.Bass()
    x_in = nc.declare_dram_parameter("x_loc", [BPC, N, D], F32, isOutput=False)
    w_in = nc.declare_dram_parameter("w_in", [D, K], F32, isOutput=False)
    c_in = nc.declare_dram_parameter("c_in", [D, K], F32, isOutput=False)
    y_out = nc.declare_dram_parameter("y_loc", [BPC, D * K], F32, isOutput=True)
    with ExitStack() as ctx:
        tc = ctx.enter_context(tile.TileContext(nc))
        tc._drain_and_barrier = types.MethodType(_split_drain_and_barrier, tc)
        _emit(ctx, tc, y_out, x_in, w_in, c_in)
    # strip same-engine self-waits from multi-wait instructions: the engines
    # dispatch in FIFO order and DVE/ACT drain between ops, so a self-wait
    # whose target precedes in the same stream guards only pseudo-hazards
    # (PSUM bank read-read serialization); walrus allows one wait only.
    eng_name = {mybir.EngineType.Activation: "Activation",
                mybir.EngineType.PE: "PE",
                mybir.EngineType.DVE: "DVE",
                mybir.EngineType.Pool: "Pool",
                mybir.EngineType.SP: "SP"}
    for name, inst in nc.inst_map.items():
        si = inst.sync_info
        if si is None or len(si.on_wait) <= 1:
            continue
        en = eng_name.get(getattr(inst, "engine", None))
        if en is None:
            continue
        keep = [w for w in si.on_wait if not w.ant_name.startswith(en + "_")]
        if isinstance(inst, mybir.InstDMACopy) and len(keep) > 1:
            # inter-DMA WAW waits on the shared SP queue guard disjoint DRAM
            # regions (one output slab per batch); queue order + disjoint
            # destinations make them redundant, and walrus allows one wait.
            keep2 = [w for w in keep if not w.ant_name.startswith("DMAHW")]
            if keep2:
                keep = keep2
        if 0 < len(keep) < len(si.on_wait):
            inst.sync_info = mybir.SyncInfo(on_wait=keep,
                                            on_update=list(si.on_update))
    return nc


def run(x, W, C, trace=False, tmpdir=None):
    from concourse.bass_utils import run_bass_kernel_spmd

    x = np.ascontiguousarray(x, dtype=np.float32).reshape(B, N, D)
    W = np.ascontiguousarray(W, dtype=np.float32)
    C = np.ascontiguousarray(C, dtype=np.float32)

    nc = build_bass()
    in_maps = [
        {"x_loc": x[c * BPC:(c + 1) * BPC], "w_in": W, "c_in": C}
        for c in range(NCORES)
    ]
    res = run_bass_kernel_spmd(nc, in_maps, list(range(NCORES)), trace=trace,
                               tmpdir=tmpdir)
    y = np.concatenate([res.results[c]["y_loc"] for c in range(NCORES)], axis=0)
    return y.astype(np.float32), res


def kernel(x, W, C):
    y, _ = run(x, W, C, trace=False)
    return y
